# revision 27
# baseline (speedup 1.0000x reference)
"""GCN encoder (GIN conv -> 2x GCN conv) on 8 Trainium2 NeuronCores.

Strategy (dst-sharded, graph-parallel, fp8-e3m4 feature-major streams):
- Nodes sharded by dst across 8 cores (12500 each); each core owns the
  segment-sums and dense math for its nodes; weights replicated.
- Self-loops ride the edge stream as synthetic (i, i) edges.
- Message slots are stored FEATURE-MAJOR as pair-tiles: partition
  k = parity*64 + feat, column = (layer_offset(st) + s)*512 + j*128 + pos
  for rank r = 2s+parity of node (supertile st, block j, row pos).
- Aggregation = 512-wide matmuls with a CONSTANT stationary operand
  (no per-pair weight churn, streams at 1 fp8 col/cycle):
    launch A: lhsT = [s1*W_gin; s1*W_gin] bf16 -> the GIN dense layer and
      the parity pair-sum are fused into the aggregation for free; PSUM
      accumulates (x_i + sum x_j) @ W_gin feature-major directly.
    launch C: lhsT = [wcat; wcat] bf16 -> the mu/lv dense layer fused
      into the aggregation of dinv-weighted h messages (h from launch A).
- Supertile pairs stack on PSUM partition halves (tile_position col 0/64)
  so the epilogue (ACT relu+bias / scale) runs at full 128-partition width.
- Epilogues are ACT-only: launch A relu+bias -> h^T out; launch C
  scale(s2)+bias -> relu on mu rows -> [mu|lv]^T out.  No per-unit
  weight swaps anywhere: the PE pipeline never flushes.
- Outputs are feature-major [128, NU*512]; the host unshards.

Two SPMD launches, host gather between them (quantize + permute only).
"""

import numpy as np
import ml_dtypes

BF16 = ml_dtypes.bfloat16
E3M4 = ml_dtypes.float8_e3m4

N = 100000
E = 1600000
COUT = 32
NCORES = 8
NPC = N // NCORES            # 12500 real nodes per core
BLK = 128
NBLK = 100                   # blocks per core
SB = 4                       # blocks per supertile (one 512-col matmul)
NST = NBLK // SB             # 25 supertiles
NU = (NST + 1) // 2          # 13 units (2 supertiles stacked; last half)
NPCP = NBLK * BLK            # 12800 padded positions per core
AMAX = 15.0                  # e3m4 target absmax (max normal 15.5)

_cache = {}


def _layer_schedule(d_blk):
    """Per-(supertile, layer) widths.  Blocks are degree-sorted
    DESCENDING, so at pair-layer s only the prefix of blocks with
    ceil(d_b/2) > s is still active -> narrower matmuls + packed columns."""
    nlay = (np.asarray(d_blk) + 1) // 2                 # layers per block
    dh = nlay.reshape(NST, SB).max(axis=1)
    widths = []                                          # widths[st][s]
    for st in range(NST):
        nb = nlay[st * SB:(st + 1) * SB]
        widths.append([int((nb > s).sum()) for s in range(int(dh[st]))])
    # column offset of each (st, layer): cumulative over w*512/4... in cols
    laycol = []                                          # laycol[st][s]
    c = 0
    stcol = [0] * (NST + 1)
    for st in range(NST):
        laycol.append([])
        for s in range(int(dh[st])):
            laycol[st].append(c)
            c += widths[st][s] * BLK
        stcol[st + 1] = c
    return dh, widths, laycol, stcol, c


def _build(d_blk, mode, has_bias):
    """One SPMD program: stream pair-tiles against a constant stationary
    [128, 64] bf16 operand (A: [s1*gin_W; s1*gin_W], C: [wcat; wcat]).
    Epilogue A: relu+bias -> h^T.  Epilogue C: scale(s2)+bias, relu on
    mu rows -> [mu|lv]^T."""
    import concourse.bacc as bacc
    import concourse.mybir as mybir
    import concourse.tile as tile

    dh, widths, laycol, stcol, totcol = _layer_schedule(d_blk)

    nc = bacc.Bacc("TRN2", target_bir_lowering=False, debug=False,
                   enable_asserts=False, num_devices=NCORES)
    slots = nc.dram_tensor("slots", [BLK, totcol], mybir.dt.float8e3,
                           kind="ExternalInput").ap()
    outT = nc.dram_tensor("outT", [BLK, NU * 512], mybir.dt.bfloat16,
                          kind="ExternalOutput").ap()
    w2D = nc.dram_tensor("W2", [BLK, 64], mybir.dt.bfloat16,
                         kind="ExternalInput").ap()
    if mode == "A":
        gbD = nc.dram_tensor("ginb2", [BLK, 1], mybir.dt.float32,
                             kind="ExternalInput").ap()
    else:
        sclD = nc.dram_tensor("scl", [BLK, 1], mybir.dt.float32,
                              kind="ExternalInput").ap()
        if has_bias:
            bcD = nc.dram_tensor("biasc", [BLK, 1], mybir.dt.float32,
                                 kind="ExternalInput").ap()

    # unit DMA geometry: unit u covers supertiles (2u, 2u+1)
    ucol0 = [stcol[min(2 * u, NST)] for u in range(NU + 1)]

    # degree-descending node order makes natural unit order largest-
    # first: PE gets a long first unit while DMA streams ahead, and the
    # tail chain lands on the smallest unit.
    units_seq = list(range(NU))

    with tile.TileContext(nc) as tc:
        with (tc.tile_pool(name="const", bufs=1) as cpool,
              tc.tile_pool(name="sl", bufs=9) as spool,
              tc.tile_pool(name="ot", bufs=4) as opool,
              tc.tile_pool(name="ps", bufs=8, space="PSUM") as ppool):
            umax = max(ucol0[u + 1] - ucol0[u] for u in range(NU))

            def load_unit(u, fine):
                """DMA one unit's slot columns; returns the SBUF tile."""
                c0, c1 = ucol0[u], ucol0[u + 1]
                t = spool.tile([BLK, umax], mybir.dt.float8e3, tag="slot")
                if fine:
                    # geometric chunks on DIFFERENT engines: DGE setups
                    # run in parallel, PE starts on the first layers ASAP
                    n = c1 - c0
                    b = 0
                    step = 1024
                    engs = [nc.sync, nc.scalar, nc.sync, nc.scalar]
                    k = 0
                    while b < n:
                        e = min(n, b + step)
                        if n - e < 1024:
                            e = n
                        engs[min(k, 3)].dma_start(
                            out=t[:, b:e], in_=slots[:, c0 + b:c0 + e])
                        b = e
                        step *= 3
                        k += 1
                else:
                    # alternate engines so DGE setups pipeline 2-wide
                    eng = nc.sync if u % 2 == 0 else nc.scalar
                    eng.dma_start(out=t[:, :c1 - c0],
                                  in_=slots[:, c0:c1])
                return t

            w2 = cpool.tile([BLK, 64], mybir.dt.bfloat16)
            nc.scalar.dma_start(out=w2[:], in_=w2D[:])
            lhs_agg = w2
            if mode == "A":
                gb = cpool.tile([BLK, 1], mybir.dt.float32)
                nc.scalar.dma_start(out=gb[:], in_=gbD[:])
            else:
                scl = cpool.tile([BLK, 1], mybir.dt.float32)
                nc.scalar.dma_start(out=scl[:], in_=sclD[:])
                if has_bias:
                    bc = cpool.tile([BLK, 1], mybir.dt.float32)
                    nc.scalar.dma_start(out=bc[:], in_=bcD[:])
            first = load_unit(units_seq[0], True)

            oggrp = {}       # group g = u//2 -> [128, 1024] bf16 tile

            def og_slot(u):
                g = u // 2
                if g not in oggrp:
                    oggrp[g] = opool.tile([BLK, 1024], mybir.dt.bfloat16,
                                          tag="og", name=f"og{g}")
                return oggrp[g][:, (u % 2) * 512:(u % 2 + 1) * 512]

            def flush_out(u):
                if u % 2 == 1 or u == NU - 1:
                    g = u // 2
                    w = 1024 if u % 2 == 1 else 512
                    nc.scalar.dma_start(out=outT[:, g * 1024:g * 1024 + w],
                                        in_=oggrp[g][:, :w])

            for ui, u in enumerate(units_seq):
                blkt = first if ui == 0 else load_unit(u, False)
                ps = ppool.tile([BLK, 512], mybir.dt.float32, space="PSUM")
                for half in range(2):
                    st = 2 * u + half
                    if st >= NST:
                        break
                    d = int(dh[st])
                    for s in range(d):
                        w = widths[st][s]
                        o = laycol[st][s] - ucol0[u]
                        nc.tensor.matmul(
                            out=ps[half * 64:(half + 1) * 64,
                                   0:w * BLK],
                            lhsT=lhs_agg[:],
                            rhs=blkt[:, o:o + w * BLK],
                            start=(s == 0),
                            stop=(s == d - 1),
                            skip_group_check=True,
                        )
                og = og_slot(u)
                if mode == "A":
                    nc.scalar.activation(og, ps[:],
                                         mybir.ActivationFunctionType.Relu,
                                         bias=gb[:], scale=1.0)
                else:
                    nc.scalar.activation(
                        og, ps[:],
                        mybir.ActivationFunctionType.Identity,
                        bias=(bc[:] if has_bias else 0.0), scale=scl[:])
                    nc.vector.tensor_scalar_max(og[0:COUT, :],
                                                og[0:COUT, :], 0.0)
                    nc.vector.tensor_scalar_max(og[64:64 + COUT, :],
                                                og[64:64 + COUT, :], 0.0)
                flush_out(u)
    nc.compile()
    from concourse.bass_interp import get_hw_module
    nc.m = get_hw_module(nc.m)
    return nc


def _prep(edge_index):
    """Shard/sort the graph; build the feature-major slot index tables."""
    src0 = np.asarray(edge_index[0], dtype=np.int64)
    dst0 = np.asarray(edge_index[1], dtype=np.int64)
    deg_in = np.bincount(dst0, minlength=N)
    dinv = (1.0 / np.sqrt(deg_in + 1.0)).astype(np.float32)
    allN = np.arange(N, dtype=np.int64)
    src = np.concatenate([src0, allN])
    dst = np.concatenate([dst0, allN])

    cores = []
    d_blk_per_core = np.zeros((NCORES, NBLK), dtype=np.int64)
    for c in range(NCORES):
        lo, hi = c * NPC, (c + 1) * NPC
        m = (dst >= lo) & (dst < hi)
        s_c = src[m]
        d_c = (dst[m] - lo).astype(np.int64)
        deg_c = np.bincount(d_c, minlength=NPC)
        order = np.argsort(-deg_c, kind="stable")     # position -> local node
                                                  # (degree DESCENDING)
        pos = np.empty(NPC, dtype=np.int64)
        pos[order] = np.arange(NPC)                   # local node -> position
        posdeg = np.zeros(NPCP, dtype=np.int64)
        posdeg[:NPC] = deg_c[order]
        d_blk_per_core[c] = posdeg.reshape(NBLK, BLK).max(axis=1)
        cores.append((s_c, d_c, order, pos, posdeg))

    d_blk = np.maximum(d_blk_per_core.max(axis=0), 2)
    dh, widths, laycol, stcol, totcol = _layer_schedule(d_blk)
    dhmax = int(dh.max())
    laycol_arr = np.zeros((NST, dhmax), dtype=np.int64)
    for st in range(NST):
        for s in range(int(dh[st])):
            laycol_arr[st, s] = laycol[st][s]

    # per-core slot index (source node id per (parity, column)) + coef
    idx = np.full((NCORES, 2, totcol), N, dtype=np.int64)
    coef = np.zeros((NCORES, 2, totcol), dtype=np.float32)
    pos_of_global = np.empty(N, dtype=np.int64)
    for c in range(NCORES):
        s_c, d_c, order, pos, posdeg = cores[c]
        pos_of_global[c * NPC + order] = c * NPCP + np.arange(NPC)
        key = pos[d_c]
        eord = np.argsort(key, kind="stable")
        spos = key[eord]                              # node position per edge
        start_of_pos = np.zeros(NPCP, dtype=np.int64)
        np.cumsum(posdeg[:-1], out=start_of_pos[1:])
        r = np.arange(len(spos)) - start_of_pos[spos]  # rank within node
        se = s_c[eord]
        de = d_c[eord] + c * NPC
        blk = spos // BLK
        row = spos % BLK
        st = blk // SB
        j = blk % SB
        s = r // 2
        col = laycol_arr[st, s] + j * BLK + row
        par = r % 2
        idx[c, par, col] = se
        coef[c, par, col] = dinv[se] * dinv[de]
    return d_blk, totcol, idx, coef, pos_of_global, cores


TRACE = False
last_exec_ns = []


def _run(nc, in_maps):
    from concourse import bass_utils
    res = bass_utils.run_bass_kernel_spmd(nc, in_maps,
                                          core_ids=list(range(NCORES)),
                                          trace=TRACE)
    if TRACE:
        last_exec_ns.append(res.exec_time_ns)
    return res.results


def _unstack(o):
    """[128, NU*512] feature-major stacked -> [NPCP, 64] position-major."""
    o = np.asarray(o, dtype=np.float32)
    top = o[0:64].reshape(64, NU, 512).transpose(1, 2, 0)      # st 0,2,..
    bot = o[64:128].reshape(64, NU, 512).transpose(1, 2, 0)    # st 1,3,..
    res = np.empty((NST, 512, 64), dtype=np.float32)
    res[0::2] = top[: (NST + 1) // 2]
    res[1::2] = bot[: NST // 2]
    return res.reshape(NPCP, 64)


def kernel(x, edge_index, gin_W, gin_b, mu_W, mu_b, lv_W, lv_b):
    x = np.asarray(x, dtype=np.float32)
    gin_W = np.asarray(gin_W, dtype=np.float32)
    gin_b = np.asarray(gin_b, dtype=np.float32)
    wcat = np.concatenate([np.asarray(mu_W, np.float32),
                           np.asarray(lv_W, np.float32)], axis=1)
    bias_cat = np.concatenate([np.asarray(mu_b, np.float32),
                               np.asarray(lv_b, np.float32)])
    has_bias = bool(np.any(bias_cat != 0))

    d_blk, totcol, idx, coef, pos_of_global, cores = _prep(edge_index)

    key = ("prog", has_bias, tuple(int(v) for v in d_blk))
    if key not in _cache:
        _cache[key] = (_build(d_blk, "A", False), _build(d_blk, "C", has_bias))
    nc_A, nc_C = _cache[key]

    # ---- launch A inputs ----
    s1 = float(np.abs(x).max()) / AMAX
    xq = np.zeros((N + 1, 64), dtype=E3M4)
    xq[:N] = (x / s1).astype(E3M4)
    W2 = np.vstack([s1 * gin_W, s1 * gin_W]).astype(BF16)
    ginb2 = np.concatenate([gin_b, gin_b]).reshape(128, 1).astype(np.float32)

    in_maps_A = []
    for c in range(NCORES):
        tbl = np.empty((BLK, totcol), dtype=E3M4)
        tbl[0:64] = xq[idx[c, 0]].T
        tbl[64:128] = xq[idx[c, 1]].T
        in_maps_A.append({
            "slots": tbl,
            "W2": W2,
            "ginb2": ginb2,
        })
    res_A = _run(nc_A, in_maps_A)

    # ---- assemble h table, build launch C inputs ----
    p_pos = np.zeros((NCORES * NPCP + 1, 64), dtype=np.float32)
    for c in range(NCORES):
        p_pos[c * NPCP:(c + 1) * NPCP] = _unstack(res_A[c]["outT"])

    gidx = np.where(idx < N + 0, pos_of_global[np.minimum(idx, N - 1)],
                    NCORES * NPCP)
    gidx[idx >= N] = NCORES * NPCP

    rowmax = np.abs(p_pos).max(axis=1)
    s2 = 0.0
    for c in range(NCORES):
        s2 = max(s2, float((coef[c] * rowmax[gidx[c]]).max()))
    s2 /= AMAX

    WC = np.vstack([wcat, wcat]).astype(BF16)
    in_maps_C = []
    for c in range(NCORES):
        tbl = np.empty((BLK, totcol), dtype=E3M4)
        for par in range(2):
            vals = p_pos[gidx[c, par]] * (coef[c, par] / s2)[:, None]
            tbl[par * 64:(par + 1) * 64] = vals.astype(E3M4).T
        im = {
            "slots": tbl,
            "W2": WC,
            "scl": np.full((BLK, 1), s2, dtype=np.float32),
        }
        if has_bias:
            im["biasc"] = np.concatenate(
                [bias_cat, bias_cat]).reshape(128, 1).astype(np.float32)
        in_maps_C.append(im)
    res_C = _run(nc_C, in_maps_C)

    # ---- unshard ----
    mu = np.empty((N, COUT), dtype=np.float32)
    lv = np.empty((N, COUT), dtype=np.float32)
    for c in range(NCORES):
        _, _, order, _, _ = cores[c]
        o = _unstack(res_C[c]["outT"])[:NPC]
        mu[c * NPC + order] = o[:, :COUT]
        lv[c * NPC + order] = o[:, COUT:]
    return mu, lv


# revision 28
# speedup vs baseline: 1.0382x; 1.0382x over previous
"""GCN encoder (GIN conv -> 2x GCN conv) on 8 Trainium2 NeuronCores.

Strategy (dst-sharded, graph-parallel, fp8-e3m4 feature-major streams):
- Nodes sharded by dst across 8 cores (12500 each); each core owns the
  segment-sums and dense math for its nodes; weights replicated.
- Self-loops ride the edge stream as synthetic (i, i) edges.
- Message slots are stored FEATURE-MAJOR as pair-tiles: partition
  k = parity*64 + feat, column = (layer_offset(st) + s)*512 + j*128 + pos
  for rank r = 2s+parity of node (supertile st, block j, row pos).
- Aggregation = 512-wide matmuls with a CONSTANT stationary operand
  (no per-pair weight churn, streams at 1 fp8 col/cycle):
    launch A: lhsT = [s1*W_gin; s1*W_gin] bf16 -> the GIN dense layer and
      the parity pair-sum are fused into the aggregation for free; PSUM
      accumulates (x_i + sum x_j) @ W_gin feature-major directly.
    launch C: lhsT = [wcat; wcat] bf16 -> the mu/lv dense layer fused
      into the aggregation of dinv-weighted h messages (h from launch A).
- Supertile pairs stack on PSUM partition halves (tile_position col 0/64)
  so the epilogue (ACT relu+bias / scale) runs at full 128-partition width.
- Epilogues are ACT-only: launch A relu+bias -> h^T out; launch C
  scale(s2)+bias -> relu on mu rows -> [mu|lv]^T out.  No per-unit
  weight swaps anywhere: the PE pipeline never flushes.
- Outputs are feature-major [128, NU*512]; the host unshards.

Two SPMD launches, host gather between them (quantize + permute only).
"""

import numpy as np
import ml_dtypes

BF16 = ml_dtypes.bfloat16
E3M4 = ml_dtypes.float8_e3m4

N = 100000
E = 1600000
COUT = 32
NCORES = 8
NPC = N // NCORES            # 12500 real nodes per core
BLK = 128
NBLK = 100                   # blocks per core
SB = 4                       # blocks per supertile (one 512-col matmul)
NST = NBLK // SB             # 25 supertiles
NU = (NST + 1) // 2          # 13 units (2 supertiles stacked; last half)
NPCP = NBLK * BLK            # 12800 padded positions per core
AMAX = 15.0                  # e3m4 target absmax (max normal 15.5)

_cache = {}


def _layer_schedule(d_blk):
    """Per-(supertile, layer) widths.  Blocks are degree-sorted
    DESCENDING, so at pair-layer s only the prefix of blocks with
    ceil(d_b/2) > s is still active -> narrower matmuls + packed columns."""
    nlay = (np.asarray(d_blk) + 1) // 2                 # layers per block
    dh = nlay.reshape(NST, SB).max(axis=1)
    widths = []                                          # widths[st][s]
    for st in range(NST):
        nb = nlay[st * SB:(st + 1) * SB]
        widths.append([int((nb > s).sum()) for s in range(int(dh[st]))])
    # column offset of each (st, layer): cumulative over w*512/4... in cols
    laycol = []                                          # laycol[st][s]
    c = 0
    stcol = [0] * (NST + 1)
    for st in range(NST):
        laycol.append([])
        for s in range(int(dh[st])):
            laycol[st].append(c)
            c += widths[st][s] * BLK
        stcol[st + 1] = c
    return dh, widths, laycol, stcol, c


def _build(d_blk, mode, has_bias):
    """One SPMD program: stream pair-tiles against a constant stationary
    [128, 64] bf16 operand (A: [s1*gin_W; s1*gin_W], C: [wcat; wcat]).
    Epilogue A: relu+bias -> h^T.  Epilogue C: scale(s2)+bias, relu on
    mu rows -> [mu|lv]^T."""
    import concourse.bacc as bacc
    import concourse.mybir as mybir
    import concourse.tile as tile

    dh, widths, laycol, stcol, totcol = _layer_schedule(d_blk)

    nc = bacc.Bacc("TRN2", target_bir_lowering=False, debug=False,
                   enable_asserts=False, num_devices=NCORES)
    slots = nc.dram_tensor("slots", [BLK, totcol], mybir.dt.float8e3,
                           kind="ExternalInput").ap()
    outT = nc.dram_tensor("outT", [BLK, NU * 512], mybir.dt.bfloat16,
                          kind="ExternalOutput").ap()
    w2D = nc.dram_tensor("W2", [BLK, 64], mybir.dt.bfloat16,
                         kind="ExternalInput").ap()
    if mode == "A":
        gbD = nc.dram_tensor("ginb2", [BLK, 1], mybir.dt.float32,
                             kind="ExternalInput").ap()
    else:
        sclD = nc.dram_tensor("scl", [BLK, 1], mybir.dt.float32,
                              kind="ExternalInput").ap()
        if has_bias:
            bcD = nc.dram_tensor("biasc", [BLK, 1], mybir.dt.float32,
                                 kind="ExternalInput").ap()

    # unit DMA geometry: unit u covers supertiles (2u, 2u+1)
    ucol0 = [stcol[min(2 * u, NST)] for u in range(NU + 1)]

    # degree-descending node order makes natural unit order largest-
    # first: PE gets a long first unit while DMA streams ahead, and the
    # tail chain lands on the smallest unit.
    units_seq = list(range(NU))

    with tile.TileContext(nc) as tc:
        with (tc.tile_pool(name="const", bufs=1) as cpool,
              tc.tile_pool(name="sl", bufs=9) as spool,
              tc.tile_pool(name="ot", bufs=4) as opool,
              tc.tile_pool(name="ps", bufs=8, space="PSUM") as ppool):
            umax = max(ucol0[u + 1] - ucol0[u] for u in range(NU))

            def load_unit(u, fine):
                """DMA one unit's slot columns; returns the SBUF tile."""
                c0, c1 = ucol0[u], ucol0[u + 1]
                t = spool.tile([BLK, umax], mybir.dt.float8e3, tag="slot")
                if fine:
                    # geometric chunks on DIFFERENT engines: DGE setups
                    # run in parallel, PE starts on the first layers ASAP
                    n = c1 - c0
                    b = 0
                    step = 1024
                    engs = [nc.sync, nc.scalar, nc.sync, nc.scalar]
                    k = 0
                    while b < n:
                        e = min(n, b + step)
                        if n - e < 1024:
                            e = n
                        engs[min(k, 3)].dma_start(
                            out=t[:, b:e], in_=slots[:, c0 + b:c0 + e])
                        b = e
                        step *= 3
                        k += 1
                else:
                    # single sync ring: strict FIFO keeps delivery in
                    # consumption order (a second ring steals bandwidth
                    # from the unit PE is actually waiting on)
                    nc.sync.dma_start(out=t[:, :c1 - c0],
                                      in_=slots[:, c0:c1])
                return t

            w2 = cpool.tile([BLK, 64], mybir.dt.bfloat16)
            nc.scalar.dma_start(out=w2[:], in_=w2D[:])
            lhs_agg = w2
            if mode == "A":
                gb = cpool.tile([BLK, 1], mybir.dt.float32)
                nc.scalar.dma_start(out=gb[:], in_=gbD[:])
            else:
                scl = cpool.tile([BLK, 1], mybir.dt.float32)
                nc.scalar.dma_start(out=scl[:], in_=sclD[:])
                if has_bias:
                    bc = cpool.tile([BLK, 1], mybir.dt.float32)
                    nc.scalar.dma_start(out=bc[:], in_=bcD[:])
            first = load_unit(units_seq[0], True)

            oggrp = {}       # group g = u//2 -> [128, 1024] bf16 tile

            def og_slot(u):
                g = u // 2
                if g not in oggrp:
                    oggrp[g] = opool.tile([BLK, 1024], mybir.dt.bfloat16,
                                          tag="og", name=f"og{g}")
                return oggrp[g][:, (u % 2) * 512:(u % 2 + 1) * 512]

            def flush_out(u):
                if u % 2 == 1 or u == NU - 1:
                    g = u // 2
                    w = 1024 if u % 2 == 1 else 512
                    nc.scalar.dma_start(out=outT[:, g * 1024:g * 1024 + w],
                                        in_=oggrp[g][:, :w])

            for ui, u in enumerate(units_seq):
                blkt = first if ui == 0 else load_unit(u, False)
                ps = ppool.tile([BLK, 512], mybir.dt.float32, space="PSUM")
                for half in range(2):
                    st = 2 * u + half
                    if st >= NST:
                        break
                    d = int(dh[st])
                    for s in range(d):
                        w = widths[st][s]
                        o = laycol[st][s] - ucol0[u]
                        nc.tensor.matmul(
                            out=ps[half * 64:(half + 1) * 64,
                                   0:w * BLK],
                            lhsT=lhs_agg[:],
                            rhs=blkt[:, o:o + w * BLK],
                            start=(s == 0),
                            stop=(s == d - 1),
                            skip_group_check=True,
                        )
                og = og_slot(u)
                if mode == "A":
                    nc.scalar.activation(og, ps[:],
                                         mybir.ActivationFunctionType.Relu,
                                         bias=gb[:], scale=1.0)
                else:
                    nc.scalar.activation(
                        og, ps[:],
                        mybir.ActivationFunctionType.Identity,
                        bias=(bc[:] if has_bias else 0.0), scale=scl[:])
                    nc.vector.tensor_scalar_max(og[0:COUT, :],
                                                og[0:COUT, :], 0.0)
                    nc.vector.tensor_scalar_max(og[64:64 + COUT, :],
                                                og[64:64 + COUT, :], 0.0)
                flush_out(u)
    nc.compile()
    from concourse.bass_interp import get_hw_module
    nc.m = get_hw_module(nc.m)
    return nc


def _prep(edge_index):
    """Shard/sort the graph; build the feature-major slot index tables."""
    src0 = np.asarray(edge_index[0], dtype=np.int64)
    dst0 = np.asarray(edge_index[1], dtype=np.int64)
    deg_in = np.bincount(dst0, minlength=N)
    dinv = (1.0 / np.sqrt(deg_in + 1.0)).astype(np.float32)
    allN = np.arange(N, dtype=np.int64)
    src = np.concatenate([src0, allN])
    dst = np.concatenate([dst0, allN])

    cores = []
    d_blk_per_core = np.zeros((NCORES, NBLK), dtype=np.int64)
    for c in range(NCORES):
        lo, hi = c * NPC, (c + 1) * NPC
        m = (dst >= lo) & (dst < hi)
        s_c = src[m]
        d_c = (dst[m] - lo).astype(np.int64)
        deg_c = np.bincount(d_c, minlength=NPC)
        order = np.argsort(-deg_c, kind="stable")     # position -> local node
                                                  # (degree DESCENDING)
        pos = np.empty(NPC, dtype=np.int64)
        pos[order] = np.arange(NPC)                   # local node -> position
        posdeg = np.zeros(NPCP, dtype=np.int64)
        posdeg[:NPC] = deg_c[order]
        d_blk_per_core[c] = posdeg.reshape(NBLK, BLK).max(axis=1)
        cores.append((s_c, d_c, order, pos, posdeg))

    d_blk = np.maximum(d_blk_per_core.max(axis=0), 2)
    dh, widths, laycol, stcol, totcol = _layer_schedule(d_blk)
    dhmax = int(dh.max())
    laycol_arr = np.zeros((NST, dhmax), dtype=np.int64)
    for st in range(NST):
        for s in range(int(dh[st])):
            laycol_arr[st, s] = laycol[st][s]

    # per-core slot index (source node id per (parity, column)) + coef
    idx = np.full((NCORES, 2, totcol), N, dtype=np.int64)
    coef = np.zeros((NCORES, 2, totcol), dtype=np.float32)
    pos_of_global = np.empty(N, dtype=np.int64)
    for c in range(NCORES):
        s_c, d_c, order, pos, posdeg = cores[c]
        pos_of_global[c * NPC + order] = c * NPCP + np.arange(NPC)
        key = pos[d_c]
        eord = np.argsort(key, kind="stable")
        spos = key[eord]                              # node position per edge
        start_of_pos = np.zeros(NPCP, dtype=np.int64)
        np.cumsum(posdeg[:-1], out=start_of_pos[1:])
        r = np.arange(len(spos)) - start_of_pos[spos]  # rank within node
        se = s_c[eord]
        de = d_c[eord] + c * NPC
        blk = spos // BLK
        row = spos % BLK
        st = blk // SB
        j = blk % SB
        s = r // 2
        col = laycol_arr[st, s] + j * BLK + row
        par = r % 2
        idx[c, par, col] = se
        coef[c, par, col] = dinv[se] * dinv[de]
    return d_blk, totcol, idx, coef, pos_of_global, cores


TRACE = False
last_exec_ns = []


def _run(nc, in_maps):
    from concourse import bass_utils
    res = bass_utils.run_bass_kernel_spmd(nc, in_maps,
                                          core_ids=list(range(NCORES)),
                                          trace=TRACE)
    if TRACE:
        last_exec_ns.append(res.exec_time_ns)
    return res.results


def _unstack(o):
    """[128, NU*512] feature-major stacked -> [NPCP, 64] position-major."""
    o = np.asarray(o, dtype=np.float32)
    top = o[0:64].reshape(64, NU, 512).transpose(1, 2, 0)      # st 0,2,..
    bot = o[64:128].reshape(64, NU, 512).transpose(1, 2, 0)    # st 1,3,..
    res = np.empty((NST, 512, 64), dtype=np.float32)
    res[0::2] = top[: (NST + 1) // 2]
    res[1::2] = bot[: NST // 2]
    return res.reshape(NPCP, 64)


def kernel(x, edge_index, gin_W, gin_b, mu_W, mu_b, lv_W, lv_b):
    x = np.asarray(x, dtype=np.float32)
    gin_W = np.asarray(gin_W, dtype=np.float32)
    gin_b = np.asarray(gin_b, dtype=np.float32)
    wcat = np.concatenate([np.asarray(mu_W, np.float32),
                           np.asarray(lv_W, np.float32)], axis=1)
    bias_cat = np.concatenate([np.asarray(mu_b, np.float32),
                               np.asarray(lv_b, np.float32)])
    has_bias = bool(np.any(bias_cat != 0))

    d_blk, totcol, idx, coef, pos_of_global, cores = _prep(edge_index)

    key = ("prog", has_bias, tuple(int(v) for v in d_blk))
    if key not in _cache:
        _cache[key] = (_build(d_blk, "A", False), _build(d_blk, "C", has_bias))
    nc_A, nc_C = _cache[key]

    # ---- launch A inputs ----
    s1 = float(np.abs(x).max()) / AMAX
    xq = np.zeros((N + 1, 64), dtype=E3M4)
    xq[:N] = (x / s1).astype(E3M4)
    W2 = np.vstack([s1 * gin_W, s1 * gin_W]).astype(BF16)
    ginb2 = np.concatenate([gin_b, gin_b]).reshape(128, 1).astype(np.float32)

    in_maps_A = []
    for c in range(NCORES):
        tbl = np.empty((BLK, totcol), dtype=E3M4)
        tbl[0:64] = xq[idx[c, 0]].T
        tbl[64:128] = xq[idx[c, 1]].T
        in_maps_A.append({
            "slots": tbl,
            "W2": W2,
            "ginb2": ginb2,
        })
    res_A = _run(nc_A, in_maps_A)

    # ---- assemble h table, build launch C inputs ----
    p_pos = np.zeros((NCORES * NPCP + 1, 64), dtype=np.float32)
    for c in range(NCORES):
        p_pos[c * NPCP:(c + 1) * NPCP] = _unstack(res_A[c]["outT"])

    gidx = np.where(idx < N + 0, pos_of_global[np.minimum(idx, N - 1)],
                    NCORES * NPCP)
    gidx[idx >= N] = NCORES * NPCP

    rowmax = np.abs(p_pos).max(axis=1)
    s2 = 0.0
    for c in range(NCORES):
        s2 = max(s2, float((coef[c] * rowmax[gidx[c]]).max()))
    s2 /= AMAX

    WC = np.vstack([wcat, wcat]).astype(BF16)
    in_maps_C = []
    for c in range(NCORES):
        tbl = np.empty((BLK, totcol), dtype=E3M4)
        for par in range(2):
            vals = p_pos[gidx[c, par]] * (coef[c, par] / s2)[:, None]
            tbl[par * 64:(par + 1) * 64] = vals.astype(E3M4).T
        im = {
            "slots": tbl,
            "W2": WC,
            "scl": np.full((BLK, 1), s2, dtype=np.float32),
        }
        if has_bias:
            im["biasc"] = np.concatenate(
                [bias_cat, bias_cat]).reshape(128, 1).astype(np.float32)
        in_maps_C.append(im)
    res_C = _run(nc_C, in_maps_C)

    # ---- unshard ----
    mu = np.empty((N, COUT), dtype=np.float32)
    lv = np.empty((N, COUT), dtype=np.float32)
    for c in range(NCORES):
        _, _, order, _, _ = cores[c]
        o = _unstack(res_C[c]["outT"])[:NPC]
        mu[c * NPC + order] = o[:, :COUT]
        lv[c * NPC + order] = o[:, COUT:]
    return mu, lv


# revision 29
# speedup vs baseline: 1.0694x; 1.0301x over previous
"""GCN encoder (GIN conv -> 2x GCN conv) on 8 Trainium2 NeuronCores.

Strategy (dst-sharded, graph-parallel, fp8-e3m4 feature-major streams):
- Nodes sharded by dst across 8 cores (12500 each); each core owns the
  segment-sums and dense math for its nodes; weights replicated.
- Self-loops ride the edge stream as synthetic (i, i) edges.
- Message slots are stored FEATURE-MAJOR as pair-tiles: partition
  k = parity*64 + feat, column = (layer_offset(st) + s)*512 + j*128 + pos
  for rank r = 2s+parity of node (supertile st, block j, row pos).
- Aggregation = 512-wide matmuls with a CONSTANT stationary operand
  (no per-pair weight churn, streams at 1 fp8 col/cycle):
    launch A: lhsT = [s1*W_gin; s1*W_gin] bf16 -> the GIN dense layer and
      the parity pair-sum are fused into the aggregation for free; PSUM
      accumulates (x_i + sum x_j) @ W_gin feature-major directly.
    launch C: lhsT = [wcat; wcat] bf16 -> the mu/lv dense layer fused
      into the aggregation of dinv-weighted h messages (h from launch A).
- Supertile pairs stack on PSUM partition halves (tile_position col 0/64)
  so the epilogue (ACT relu+bias / scale) runs at full 128-partition width.
- Epilogues are ACT-only: launch A relu+bias -> h^T out; launch C
  scale(s2)+bias -> relu on mu rows -> [mu|lv]^T out.  No per-unit
  weight swaps anywhere: the PE pipeline never flushes.
- Outputs are feature-major [128, NU*512]; the host unshards.

Two SPMD launches, host gather between them (quantize + permute only).
"""

import numpy as np
import ml_dtypes

BF16 = ml_dtypes.bfloat16
E3M4 = ml_dtypes.float8_e3m4

N = 100000
E = 1600000
COUT = 32
NCORES = 8
NPC = N // NCORES            # 12500 real nodes per core
BLK = 128
NBLK = 100                   # blocks per core
SB = 4                       # blocks per supertile (one 512-col matmul)
NST = NBLK // SB             # 25 supertiles
NU = (NST + 1) // 2          # 13 units (2 supertiles stacked; last half)
NPCP = NBLK * BLK            # 12800 padded positions per core
AMAX = 15.0                  # e3m4 target absmax (max normal 15.5)

_cache = {}


def _layer_schedule(d_blk):
    """Per-(supertile, layer) widths.  Blocks are degree-sorted
    DESCENDING, so at pair-layer s only the prefix of blocks with
    ceil(d_b/2) > s is still active -> narrower matmuls + packed columns."""
    nlay = (np.asarray(d_blk) + 1) // 2                 # layers per block
    dh = nlay.reshape(NST, SB).max(axis=1)
    widths = []                                          # widths[st][s]
    for st in range(NST):
        nb = nlay[st * SB:(st + 1) * SB]
        widths.append([int((nb > s).sum()) for s in range(int(dh[st]))])
    # column offset of each (st, layer): cumulative over w*512/4... in cols
    laycol = []                                          # laycol[st][s]
    c = 0
    stcol = [0] * (NST + 1)
    for st in range(NST):
        laycol.append([])
        for s in range(int(dh[st])):
            laycol[st].append(c)
            c += widths[st][s] * BLK
        stcol[st + 1] = c
    return dh, widths, laycol, stcol, c


def _build(d_blk, mode, has_bias):
    """One SPMD program: stream pair-tiles against a constant stationary
    [128, 64] bf16 operand (A: [s1*gin_W; s1*gin_W], C: [wcat; wcat]).
    Epilogue A: relu+bias -> h^T.  Epilogue C: scale(s2)+bias, relu on
    mu rows -> [mu|lv]^T."""
    import concourse.bacc as bacc
    import concourse.mybir as mybir
    import concourse.tile as tile

    dh, widths, laycol, stcol, totcol = _layer_schedule(d_blk)

    nc = bacc.Bacc("TRN2", target_bir_lowering=False, debug=False,
                   enable_asserts=False, num_devices=NCORES)
    slots = nc.dram_tensor("slots", [BLK, totcol], mybir.dt.float8e3,
                           kind="ExternalInput").ap()
    outT = nc.dram_tensor("outT", [BLK, NU * 512], mybir.dt.bfloat16,
                          kind="ExternalOutput").ap()
    w2D = nc.dram_tensor("W2", [BLK, 64], mybir.dt.bfloat16,
                         kind="ExternalInput").ap()
    if mode == "A":
        gbD = nc.dram_tensor("ginb2", [BLK, 1], mybir.dt.float32,
                             kind="ExternalInput").ap()
    else:
        sclD = nc.dram_tensor("scl", [BLK, 1], mybir.dt.float32,
                              kind="ExternalInput").ap()
        if has_bias:
            bcD = nc.dram_tensor("biasc", [BLK, 1], mybir.dt.float32,
                                 kind="ExternalInput").ap()

    # unit DMA geometry: unit u covers supertiles (2u, 2u+1)
    ucol0 = [stcol[min(2 * u, NST)] for u in range(NU + 1)]

    # degree-descending node order makes natural unit order largest-
    # first: PE gets a long first unit while DMA streams ahead, and the
    # tail chain lands on the smallest unit.
    units_seq = list(range(NU))

    with tile.TileContext(nc) as tc:
        with (tc.tile_pool(name="const", bufs=1) as cpool,
              tc.tile_pool(name="sl", bufs=9) as spool,
              tc.tile_pool(name="ot", bufs=4) as opool,
              tc.tile_pool(name="ps", bufs=8, space="PSUM") as ppool):
            umax = max(ucol0[u + 1] - ucol0[u] for u in range(NU))

            def load_unit(u, fine):
                """DMA one unit's slot columns; returns the SBUF tile."""
                c0, c1 = ucol0[u], ucol0[u + 1]
                t = spool.tile([BLK, umax], mybir.dt.float8e3, tag="slot")
                if fine:
                    # geometric chunks, ALL on the sync ring: FIFO delivers
                    # chunk1 first so PE starts early, and no later chunk
                    # ends up sharing queue grants with prefetch of the
                    # next units on a second ring
                    n = c1 - c0
                    b = 0
                    step = 1024
                    while b < n:
                        e = min(n, b + step)
                        if n - e < 1024:
                            e = n
                        nc.sync.dma_start(
                            out=t[:, b:e], in_=slots[:, c0 + b:c0 + e])
                        b = e
                        step *= 3
                else:
                    # single sync ring: strict FIFO keeps delivery in
                    # consumption order (a second ring steals bandwidth
                    # from the unit PE is actually waiting on)
                    nc.sync.dma_start(out=t[:, :c1 - c0],
                                      in_=slots[:, c0:c1])
                return t

            w2 = cpool.tile([BLK, 64], mybir.dt.bfloat16)
            nc.scalar.dma_start(out=w2[:], in_=w2D[:])
            lhs_agg = w2
            if mode == "A":
                gb = cpool.tile([BLK, 1], mybir.dt.float32)
                nc.scalar.dma_start(out=gb[:], in_=gbD[:])
            else:
                scl = cpool.tile([BLK, 1], mybir.dt.float32)
                nc.scalar.dma_start(out=scl[:], in_=sclD[:])
                if has_bias:
                    bc = cpool.tile([BLK, 1], mybir.dt.float32)
                    nc.scalar.dma_start(out=bc[:], in_=bcD[:])
            first = load_unit(units_seq[0], True)

            oggrp = {}       # group g = u//2 -> [128, 1024] bf16 tile

            def og_slot(u):
                g = u // 2
                if g not in oggrp:
                    oggrp[g] = opool.tile([BLK, 1024], mybir.dt.bfloat16,
                                          tag="og", name=f"og{g}")
                return oggrp[g][:, (u % 2) * 512:(u % 2 + 1) * 512]

            def flush_out(u):
                if u % 2 == 1 or u == NU - 1:
                    g = u // 2
                    w = 1024 if u % 2 == 1 else 512
                    nc.scalar.dma_start(out=outT[:, g * 1024:g * 1024 + w],
                                        in_=oggrp[g][:, :w])

            for ui, u in enumerate(units_seq):
                blkt = first if ui == 0 else load_unit(u, False)
                ps = ppool.tile([BLK, 512], mybir.dt.float32, space="PSUM")
                for half in range(2):
                    st = 2 * u + half
                    if st >= NST:
                        break
                    d = int(dh[st])
                    for s in range(d):
                        w = widths[st][s]
                        o = laycol[st][s] - ucol0[u]
                        nc.tensor.matmul(
                            out=ps[half * 64:(half + 1) * 64,
                                   0:w * BLK],
                            lhsT=lhs_agg[:],
                            rhs=blkt[:, o:o + w * BLK],
                            start=(s == 0),
                            stop=(s == d - 1),
                            skip_group_check=True,
                        )
                og = og_slot(u)
                if mode == "A":
                    nc.scalar.activation(og, ps[:],
                                         mybir.ActivationFunctionType.Relu,
                                         bias=gb[:], scale=1.0)
                else:
                    nc.scalar.activation(
                        og, ps[:],
                        mybir.ActivationFunctionType.Identity,
                        bias=(bc[:] if has_bias else 0.0), scale=scl[:])
                    nc.vector.tensor_scalar_max(og[0:COUT, :],
                                                og[0:COUT, :], 0.0)
                    nc.vector.tensor_scalar_max(og[64:64 + COUT, :],
                                                og[64:64 + COUT, :], 0.0)
                flush_out(u)
    nc.compile()
    from concourse.bass_interp import get_hw_module
    nc.m = get_hw_module(nc.m)
    return nc


def _prep(edge_index):
    """Shard/sort the graph; build the feature-major slot index tables."""
    src0 = np.asarray(edge_index[0], dtype=np.int64)
    dst0 = np.asarray(edge_index[1], dtype=np.int64)
    deg_in = np.bincount(dst0, minlength=N)
    dinv = (1.0 / np.sqrt(deg_in + 1.0)).astype(np.float32)
    allN = np.arange(N, dtype=np.int64)
    src = np.concatenate([src0, allN])
    dst = np.concatenate([dst0, allN])

    cores = []
    d_blk_per_core = np.zeros((NCORES, NBLK), dtype=np.int64)
    for c in range(NCORES):
        lo, hi = c * NPC, (c + 1) * NPC
        m = (dst >= lo) & (dst < hi)
        s_c = src[m]
        d_c = (dst[m] - lo).astype(np.int64)
        deg_c = np.bincount(d_c, minlength=NPC)
        order = np.argsort(-deg_c, kind="stable")     # position -> local node
                                                  # (degree DESCENDING)
        pos = np.empty(NPC, dtype=np.int64)
        pos[order] = np.arange(NPC)                   # local node -> position
        posdeg = np.zeros(NPCP, dtype=np.int64)
        posdeg[:NPC] = deg_c[order]
        d_blk_per_core[c] = posdeg.reshape(NBLK, BLK).max(axis=1)
        cores.append((s_c, d_c, order, pos, posdeg))

    d_blk = np.maximum(d_blk_per_core.max(axis=0), 2)
    dh, widths, laycol, stcol, totcol = _layer_schedule(d_blk)
    dhmax = int(dh.max())
    laycol_arr = np.zeros((NST, dhmax), dtype=np.int64)
    for st in range(NST):
        for s in range(int(dh[st])):
            laycol_arr[st, s] = laycol[st][s]

    # per-core slot index (source node id per (parity, column)) + coef
    idx = np.full((NCORES, 2, totcol), N, dtype=np.int64)
    coef = np.zeros((NCORES, 2, totcol), dtype=np.float32)
    pos_of_global = np.empty(N, dtype=np.int64)
    for c in range(NCORES):
        s_c, d_c, order, pos, posdeg = cores[c]
        pos_of_global[c * NPC + order] = c * NPCP + np.arange(NPC)
        key = pos[d_c]
        eord = np.argsort(key, kind="stable")
        spos = key[eord]                              # node position per edge
        start_of_pos = np.zeros(NPCP, dtype=np.int64)
        np.cumsum(posdeg[:-1], out=start_of_pos[1:])
        r = np.arange(len(spos)) - start_of_pos[spos]  # rank within node
        se = s_c[eord]
        de = d_c[eord] + c * NPC
        blk = spos // BLK
        row = spos % BLK
        st = blk // SB
        j = blk % SB
        s = r // 2
        col = laycol_arr[st, s] + j * BLK + row
        par = r % 2
        idx[c, par, col] = se
        coef[c, par, col] = dinv[se] * dinv[de]
    return d_blk, totcol, idx, coef, pos_of_global, cores


TRACE = False
last_exec_ns = []


def _run(nc, in_maps):
    from concourse import bass_utils
    res = bass_utils.run_bass_kernel_spmd(nc, in_maps,
                                          core_ids=list(range(NCORES)),
                                          trace=TRACE)
    if TRACE:
        last_exec_ns.append(res.exec_time_ns)
    return res.results


def _unstack(o):
    """[128, NU*512] feature-major stacked -> [NPCP, 64] position-major."""
    o = np.asarray(o, dtype=np.float32)
    top = o[0:64].reshape(64, NU, 512).transpose(1, 2, 0)      # st 0,2,..
    bot = o[64:128].reshape(64, NU, 512).transpose(1, 2, 0)    # st 1,3,..
    res = np.empty((NST, 512, 64), dtype=np.float32)
    res[0::2] = top[: (NST + 1) // 2]
    res[1::2] = bot[: NST // 2]
    return res.reshape(NPCP, 64)


def kernel(x, edge_index, gin_W, gin_b, mu_W, mu_b, lv_W, lv_b):
    x = np.asarray(x, dtype=np.float32)
    gin_W = np.asarray(gin_W, dtype=np.float32)
    gin_b = np.asarray(gin_b, dtype=np.float32)
    wcat = np.concatenate([np.asarray(mu_W, np.float32),
                           np.asarray(lv_W, np.float32)], axis=1)
    bias_cat = np.concatenate([np.asarray(mu_b, np.float32),
                               np.asarray(lv_b, np.float32)])
    has_bias = bool(np.any(bias_cat != 0))

    d_blk, totcol, idx, coef, pos_of_global, cores = _prep(edge_index)

    key = ("prog", has_bias, tuple(int(v) for v in d_blk))
    if key not in _cache:
        _cache[key] = (_build(d_blk, "A", False), _build(d_blk, "C", has_bias))
    nc_A, nc_C = _cache[key]

    # ---- launch A inputs ----
    s1 = float(np.abs(x).max()) / AMAX
    xq = np.zeros((N + 1, 64), dtype=E3M4)
    xq[:N] = (x / s1).astype(E3M4)
    W2 = np.vstack([s1 * gin_W, s1 * gin_W]).astype(BF16)
    ginb2 = np.concatenate([gin_b, gin_b]).reshape(128, 1).astype(np.float32)

    in_maps_A = []
    for c in range(NCORES):
        tbl = np.empty((BLK, totcol), dtype=E3M4)
        tbl[0:64] = xq[idx[c, 0]].T
        tbl[64:128] = xq[idx[c, 1]].T
        in_maps_A.append({
            "slots": tbl,
            "W2": W2,
            "ginb2": ginb2,
        })
    res_A = _run(nc_A, in_maps_A)

    # ---- assemble h table, build launch C inputs ----
    p_pos = np.zeros((NCORES * NPCP + 1, 64), dtype=np.float32)
    for c in range(NCORES):
        p_pos[c * NPCP:(c + 1) * NPCP] = _unstack(res_A[c]["outT"])

    gidx = np.where(idx < N + 0, pos_of_global[np.minimum(idx, N - 1)],
                    NCORES * NPCP)
    gidx[idx >= N] = NCORES * NPCP

    rowmax = np.abs(p_pos).max(axis=1)
    s2 = 0.0
    for c in range(NCORES):
        s2 = max(s2, float((coef[c] * rowmax[gidx[c]]).max()))
    s2 /= AMAX

    WC = np.vstack([wcat, wcat]).astype(BF16)
    in_maps_C = []
    for c in range(NCORES):
        tbl = np.empty((BLK, totcol), dtype=E3M4)
        for par in range(2):
            vals = p_pos[gidx[c, par]] * (coef[c, par] / s2)[:, None]
            tbl[par * 64:(par + 1) * 64] = vals.astype(E3M4).T
        im = {
            "slots": tbl,
            "W2": WC,
            "scl": np.full((BLK, 1), s2, dtype=np.float32),
        }
        if has_bias:
            im["biasc"] = np.concatenate(
                [bias_cat, bias_cat]).reshape(128, 1).astype(np.float32)
        in_maps_C.append(im)
    res_C = _run(nc_C, in_maps_C)

    # ---- unshard ----
    mu = np.empty((N, COUT), dtype=np.float32)
    lv = np.empty((N, COUT), dtype=np.float32)
    for c in range(NCORES):
        _, _, order, _, _ = cores[c]
        o = _unstack(res_C[c]["outT"])[:NPC]
        mu[c * NPC + order] = o[:, :COUT]
        lv[c * NPC + order] = o[:, COUT:]
    return mu, lv


# revision 30
# speedup vs baseline: 1.1260x; 1.0529x over previous
"""GCN encoder (GIN conv -> 2x GCN conv) on 8 Trainium2 NeuronCores.

Strategy (dst-sharded, graph-parallel, fp8-e3m4 feature-major streams):
- Nodes sharded by dst across 8 cores (12500 each); each core owns the
  segment-sums and dense math for its nodes; weights replicated.
- Self-loops ride the edge stream as synthetic (i, i) edges.
- Message slots are stored FEATURE-MAJOR as pair-tiles: partition
  k = parity*64 + feat, column = (layer_offset(st) + s)*512 + j*128 + pos
  for rank r = 2s+parity of node (supertile st, block j, row pos).
- Aggregation = 512-wide matmuls with a CONSTANT stationary operand
  (no per-pair weight churn, streams at 1 fp8 col/cycle):
    launch A: lhsT = [s1*W_gin; s1*W_gin] bf16 -> the GIN dense layer and
      the parity pair-sum are fused into the aggregation for free; PSUM
      accumulates (x_i + sum x_j) @ W_gin feature-major directly.
    launch C: lhsT = [wcat; wcat] bf16 -> the mu/lv dense layer fused
      into the aggregation of dinv-weighted h messages (h from launch A).
- Supertile pairs stack on PSUM partition halves (tile_position col 0/64)
  so the epilogue (ACT relu+bias / scale) runs at full 128-partition width.
- Epilogues are ACT-only: launch A relu+bias -> h^T out; launch C
  scale(s2)+bias -> relu on mu rows -> [mu|lv]^T out.  No per-unit
  weight swaps anywhere: the PE pipeline never flushes.
- Outputs are feature-major [128, NU*512]; the host unshards.

Two SPMD launches, host gather between them (quantize + permute only).
"""

import numpy as np
import ml_dtypes

BF16 = ml_dtypes.bfloat16
E3M4 = ml_dtypes.float8_e3m4

N = 100000
E = 1600000
COUT = 32
NCORES = 8
NPC = N // NCORES            # 12500 real nodes per core
BLK = 128
NBLK = 100                   # blocks per core
SB = 4                       # blocks per supertile (one 512-col matmul)
NST = NBLK // SB             # 25 supertiles
NU = (NST + 1) // 2          # 13 units (2 supertiles stacked; last half)
NPCP = NBLK * BLK            # 12800 padded positions per core
AMAX = 15.0                  # e3m4 target absmax (max normal 15.5)

_cache = {}


def _layer_schedule(d_blk):
    """Per-(supertile, layer) widths.  Blocks are degree-sorted
    DESCENDING, so at pair-layer s only the prefix of blocks with
    ceil(d_b/2) > s is still active -> narrower matmuls + packed columns."""
    nlay = (np.asarray(d_blk) + 1) // 2                 # layers per block
    dh = nlay.reshape(NST, SB).max(axis=1)
    widths = []                                          # widths[st][s]
    for st in range(NST):
        nb = nlay[st * SB:(st + 1) * SB]
        widths.append([int((nb > s).sum()) for s in range(int(dh[st]))])
    # column offset of each (st, layer): cumulative over w*512/4... in cols
    laycol = []                                          # laycol[st][s]
    c = 0
    stcol = [0] * (NST + 1)
    for st in range(NST):
        laycol.append([])
        for s in range(int(dh[st])):
            laycol[st].append(c)
            c += widths[st][s] * BLK
        stcol[st + 1] = c
    return dh, widths, laycol, stcol, c


def _build(d_blk, mode, has_bias):
    """One SPMD program: stream pair-tiles against a constant stationary
    [128, 64] bf16 operand (A: [s1*gin_W; s1*gin_W], C: [wcat; wcat]).
    Epilogue A: relu+bias -> h^T.  Epilogue C: scale(s2)+bias, relu on
    mu rows -> [mu|lv]^T."""
    import concourse.bacc as bacc
    import concourse.mybir as mybir
    import concourse.tile as tile

    dh, widths, laycol, stcol, totcol = _layer_schedule(d_blk)

    nc = bacc.Bacc("TRN2", target_bir_lowering=False, debug=False,
                   enable_asserts=False, num_devices=NCORES)
    slots = nc.dram_tensor("slots", [BLK, totcol], mybir.dt.float8e3,
                           kind="ExternalInput").ap()
    outT = nc.dram_tensor("outT", [BLK, NU * 512], mybir.dt.bfloat16,
                          kind="ExternalOutput").ap()
    w2D = nc.dram_tensor("W2", [BLK, 64], mybir.dt.bfloat16,
                         kind="ExternalInput").ap()
    if mode == "A":
        gbD = nc.dram_tensor("ginb2", [BLK, 1], mybir.dt.float32,
                             kind="ExternalInput").ap()
    else:
        sclD = nc.dram_tensor("scl", [BLK, 1], mybir.dt.float32,
                              kind="ExternalInput").ap()
        if has_bias:
            bcD = nc.dram_tensor("biasc", [BLK, 1], mybir.dt.float32,
                                 kind="ExternalInput").ap()

    # unit DMA geometry: unit u covers supertiles (2u, 2u+1)
    ucol0 = [stcol[min(2 * u, NST)] for u in range(NU + 1)]

    # degree-descending node order makes natural unit order largest-
    # first: PE gets a long first unit while DMA streams ahead, and the
    # tail chain lands on the smallest unit.
    units_seq = list(range(NU))

    with tile.TileContext(nc) as tc:
        with (tc.tile_pool(name="const", bufs=1) as cpool,
              tc.tile_pool(name="sl", bufs=10) as spool,
              tc.tile_pool(name="ot", bufs=4) as opool,
              tc.tile_pool(name="ps", bufs=8, space="PSUM") as ppool):
            umax = max(ucol0[u + 1] - ucol0[u] for u in range(NU))

            def load_unit(u, fine):
                """DMA one unit's slot columns; returns the SBUF tile."""
                c0, c1 = ucol0[u], ucol0[u + 1]
                t = spool.tile([BLK, umax], mybir.dt.float8e3, tag="slot")
                if fine:
                    # geometric chunks, ALL on the sync ring: FIFO delivers
                    # chunk1 first so PE starts early, and no later chunk
                    # ends up sharing queue grants with prefetch of the
                    # next units on a second ring
                    n = c1 - c0
                    b = 0
                    step = 512
                    while b < n:
                        e = min(n, b + step)
                        if n - e < 1024:
                            e = n
                        nc.sync.dma_start(
                            out=t[:, b:e], in_=slots[:, c0 + b:c0 + e])
                        b = e
                        step *= 3
                else:
                    # single sync ring: strict FIFO keeps delivery in
                    # consumption order (a second ring steals bandwidth
                    # from the unit PE is actually waiting on)
                    nc.sync.dma_start(out=t[:, :c1 - c0],
                                      in_=slots[:, c0:c1])
                return t

            w2 = cpool.tile([BLK, 64], mybir.dt.bfloat16)
            nc.scalar.dma_start(out=w2[:], in_=w2D[:])
            lhs_agg = w2
            if mode == "A":
                gb = cpool.tile([BLK, 1], mybir.dt.float32)
                nc.scalar.dma_start(out=gb[:], in_=gbD[:])
            else:
                scl = cpool.tile([BLK, 1], mybir.dt.float32)
                nc.scalar.dma_start(out=scl[:], in_=sclD[:])
                if has_bias:
                    bc = cpool.tile([BLK, 1], mybir.dt.float32)
                    nc.scalar.dma_start(out=bc[:], in_=bcD[:])
            first = load_unit(units_seq[0], True)

            oggrp = {}       # group g = u//2 -> [128, 1024] bf16 tile

            def og_slot(u):
                g = u // 2
                if g not in oggrp:
                    oggrp[g] = opool.tile([BLK, 1024], mybir.dt.bfloat16,
                                          tag="og", name=f"og{g}")
                return oggrp[g][:, (u % 2) * 512:(u % 2 + 1) * 512]

            def flush_out(u):
                if u % 2 == 1 or u == NU - 1:
                    g = u // 2
                    w = 1024 if u % 2 == 1 else 512
                    nc.scalar.dma_start(out=outT[:, g * 1024:g * 1024 + w],
                                        in_=oggrp[g][:, :w])

            for ui, u in enumerate(units_seq):
                blkt = first if ui == 0 else load_unit(u, False)
                ps = ppool.tile([BLK, 512], mybir.dt.float32, space="PSUM")
                for half in range(2):
                    st = 2 * u + half
                    if st >= NST:
                        break
                    d = int(dh[st])
                    for s in range(d):
                        w = widths[st][s]
                        o = laycol[st][s] - ucol0[u]
                        nc.tensor.matmul(
                            out=ps[half * 64:(half + 1) * 64,
                                   0:w * BLK],
                            lhsT=lhs_agg[:],
                            rhs=blkt[:, o:o + w * BLK],
                            start=(s == 0),
                            stop=(s == d - 1),
                            skip_group_check=True,
                        )
                og = og_slot(u)
                if mode == "A":
                    nc.scalar.activation(og, ps[:],
                                         mybir.ActivationFunctionType.Relu,
                                         bias=gb[:], scale=1.0)
                else:
                    nc.scalar.activation(
                        og, ps[:],
                        mybir.ActivationFunctionType.Identity,
                        bias=(bc[:] if has_bias else 0.0), scale=scl[:])
                    nc.vector.tensor_scalar_max(og[0:COUT, :],
                                                og[0:COUT, :], 0.0)
                    nc.vector.tensor_scalar_max(og[64:64 + COUT, :],
                                                og[64:64 + COUT, :], 0.0)
                flush_out(u)
    nc.compile()
    from concourse.bass_interp import get_hw_module
    nc.m = get_hw_module(nc.m)
    return nc


def _prep(edge_index):
    """Shard/sort the graph; build the feature-major slot index tables."""
    src0 = np.asarray(edge_index[0], dtype=np.int64)
    dst0 = np.asarray(edge_index[1], dtype=np.int64)
    deg_in = np.bincount(dst0, minlength=N)
    dinv = (1.0 / np.sqrt(deg_in + 1.0)).astype(np.float32)
    allN = np.arange(N, dtype=np.int64)
    src = np.concatenate([src0, allN])
    dst = np.concatenate([dst0, allN])

    cores = []
    d_blk_per_core = np.zeros((NCORES, NBLK), dtype=np.int64)
    for c in range(NCORES):
        lo, hi = c * NPC, (c + 1) * NPC
        m = (dst >= lo) & (dst < hi)
        s_c = src[m]
        d_c = (dst[m] - lo).astype(np.int64)
        deg_c = np.bincount(d_c, minlength=NPC)
        order = np.argsort(-deg_c, kind="stable")     # position -> local node
                                                  # (degree DESCENDING)
        pos = np.empty(NPC, dtype=np.int64)
        pos[order] = np.arange(NPC)                   # local node -> position
        posdeg = np.zeros(NPCP, dtype=np.int64)
        posdeg[:NPC] = deg_c[order]
        d_blk_per_core[c] = posdeg.reshape(NBLK, BLK).max(axis=1)
        cores.append((s_c, d_c, order, pos, posdeg))

    d_blk = np.maximum(d_blk_per_core.max(axis=0), 2)
    dh, widths, laycol, stcol, totcol = _layer_schedule(d_blk)
    dhmax = int(dh.max())
    laycol_arr = np.zeros((NST, dhmax), dtype=np.int64)
    for st in range(NST):
        for s in range(int(dh[st])):
            laycol_arr[st, s] = laycol[st][s]

    # per-core slot index (source node id per (parity, column)) + coef
    idx = np.full((NCORES, 2, totcol), N, dtype=np.int64)
    coef = np.zeros((NCORES, 2, totcol), dtype=np.float32)
    pos_of_global = np.empty(N, dtype=np.int64)
    for c in range(NCORES):
        s_c, d_c, order, pos, posdeg = cores[c]
        pos_of_global[c * NPC + order] = c * NPCP + np.arange(NPC)
        key = pos[d_c]
        eord = np.argsort(key, kind="stable")
        spos = key[eord]                              # node position per edge
        start_of_pos = np.zeros(NPCP, dtype=np.int64)
        np.cumsum(posdeg[:-1], out=start_of_pos[1:])
        r = np.arange(len(spos)) - start_of_pos[spos]  # rank within node
        se = s_c[eord]
        de = d_c[eord] + c * NPC
        blk = spos // BLK
        row = spos % BLK
        st = blk // SB
        j = blk % SB
        s = r // 2
        col = laycol_arr[st, s] + j * BLK + row
        par = r % 2
        idx[c, par, col] = se
        coef[c, par, col] = dinv[se] * dinv[de]
    return d_blk, totcol, idx, coef, pos_of_global, cores


TRACE = False
last_exec_ns = []


def _run(nc, in_maps):
    from concourse import bass_utils
    res = bass_utils.run_bass_kernel_spmd(nc, in_maps,
                                          core_ids=list(range(NCORES)),
                                          trace=TRACE)
    if TRACE:
        last_exec_ns.append(res.exec_time_ns)
    return res.results


def _unstack(o):
    """[128, NU*512] feature-major stacked -> [NPCP, 64] position-major."""
    o = np.asarray(o, dtype=np.float32)
    top = o[0:64].reshape(64, NU, 512).transpose(1, 2, 0)      # st 0,2,..
    bot = o[64:128].reshape(64, NU, 512).transpose(1, 2, 0)    # st 1,3,..
    res = np.empty((NST, 512, 64), dtype=np.float32)
    res[0::2] = top[: (NST + 1) // 2]
    res[1::2] = bot[: NST // 2]
    return res.reshape(NPCP, 64)


def kernel(x, edge_index, gin_W, gin_b, mu_W, mu_b, lv_W, lv_b):
    x = np.asarray(x, dtype=np.float32)
    gin_W = np.asarray(gin_W, dtype=np.float32)
    gin_b = np.asarray(gin_b, dtype=np.float32)
    wcat = np.concatenate([np.asarray(mu_W, np.float32),
                           np.asarray(lv_W, np.float32)], axis=1)
    bias_cat = np.concatenate([np.asarray(mu_b, np.float32),
                               np.asarray(lv_b, np.float32)])
    has_bias = bool(np.any(bias_cat != 0))

    d_blk, totcol, idx, coef, pos_of_global, cores = _prep(edge_index)

    key = ("prog", has_bias, tuple(int(v) for v in d_blk))
    if key not in _cache:
        _cache[key] = (_build(d_blk, "A", False), _build(d_blk, "C", has_bias))
    nc_A, nc_C = _cache[key]

    # ---- launch A inputs ----
    s1 = float(np.abs(x).max()) / AMAX
    xq = np.zeros((N + 1, 64), dtype=E3M4)
    xq[:N] = (x / s1).astype(E3M4)
    W2 = np.vstack([s1 * gin_W, s1 * gin_W]).astype(BF16)
    ginb2 = np.concatenate([gin_b, gin_b]).reshape(128, 1).astype(np.float32)

    in_maps_A = []
    for c in range(NCORES):
        tbl = np.empty((BLK, totcol), dtype=E3M4)
        tbl[0:64] = xq[idx[c, 0]].T
        tbl[64:128] = xq[idx[c, 1]].T
        in_maps_A.append({
            "slots": tbl,
            "W2": W2,
            "ginb2": ginb2,
        })
    res_A = _run(nc_A, in_maps_A)

    # ---- assemble h table, build launch C inputs ----
    p_pos = np.zeros((NCORES * NPCP + 1, 64), dtype=np.float32)
    for c in range(NCORES):
        p_pos[c * NPCP:(c + 1) * NPCP] = _unstack(res_A[c]["outT"])

    gidx = np.where(idx < N + 0, pos_of_global[np.minimum(idx, N - 1)],
                    NCORES * NPCP)
    gidx[idx >= N] = NCORES * NPCP

    rowmax = np.abs(p_pos).max(axis=1)
    s2 = 0.0
    for c in range(NCORES):
        s2 = max(s2, float((coef[c] * rowmax[gidx[c]]).max()))
    s2 /= AMAX

    WC = np.vstack([wcat, wcat]).astype(BF16)
    in_maps_C = []
    for c in range(NCORES):
        tbl = np.empty((BLK, totcol), dtype=E3M4)
        for par in range(2):
            vals = p_pos[gidx[c, par]] * (coef[c, par] / s2)[:, None]
            tbl[par * 64:(par + 1) * 64] = vals.astype(E3M4).T
        im = {
            "slots": tbl,
            "W2": WC,
            "scl": np.full((BLK, 1), s2, dtype=np.float32),
        }
        if has_bias:
            im["biasc"] = np.concatenate(
                [bias_cat, bias_cat]).reshape(128, 1).astype(np.float32)
        in_maps_C.append(im)
    res_C = _run(nc_C, in_maps_C)

    # ---- unshard ----
    mu = np.empty((N, COUT), dtype=np.float32)
    lv = np.empty((N, COUT), dtype=np.float32)
    for c in range(NCORES):
        _, _, order, _, _ = cores[c]
        o = _unstack(res_C[c]["outT"])[:NPC]
        mu[c * NPC + order] = o[:, :COUT]
        lv[c * NPC + order] = o[:, COUT:]
    return mu, lv
